# revision 26
# baseline (speedup 1.0000x reference)
"""Gated multi-head attention (RMSNorm + RoPE + SDPA + sigmoid head gates + out-proj)
as a Trainium2 Bass/Tile kernel, data-parallel over batch across 8 NeuronCores.

Problem shapes (hardcoded): b=8, n=1024, dim=512, heads=8, dim_head=64, theta=1e4.
Each core processes one batch element; no collectives needed.

v4: dual-bank matmul blocks + wide bf16 DVE ops + head-split last window.

The mid-body is paced by the tensor engine (scores + AV + projections), with
the scalar engine's exp of the n^2 score matrix (64 tiles of [128,1024]) a
close second. v4 cuts PE time by sharing each stationary across two 512-wide
matmuls into the two banks of a [128,1024] PSUM tile (halves LDWEIGHTS on the
qk and AV paths) and cuts DVE time by doing RoPE/gating arithmetic at free
size 1024 with bf16 operands in SBUF (2x/4x DVE modes).

Schedule: 4 pair-windows of 8 beats (beat = S^T tiles for one key chunk,
both heads, + their exps). Window w also carries: AV dual-passes of pair w-1
(hh0 at beats 0-3, hh1 at 4-7), qk projections of pair w+1, and the
recip/gate-broadcast postprocessing of pair w-2. The last window is split by
head instead: sub-window A computes all hh0 scores (AV of pair 2 as filler),
sub-window B computes hh1 scores with pair 3's hh0 AV overlapped, so only
the hh1 AV pass remains for the tail.

Engine roles:
  ACT: RMS square/sqrt, half the transpose drains, gates exp (sigmoid via
       1/(1+e^-z)), 64x exp, half the tail out-copies
  DVE: RMS recip+scale, other drains, RoPE shuffle/mul/add, AV psum drains,
       denominator recip, gate scale, half the out-copies
  Pool(gpsimd): cos/sin/wg/wo/identity SWDGE loads, w_o cast, RoPE sin mul,
       v_aug ones memset (gpsimd never touches PSUM)
  SP:  x/w_qkv/gamma loads, denominator row moves, output stores
PSUM: 2x[128,1024] score tiles (4 banks; also transpose staging) +
2x[128,1024] utility tiles (4 banks) shared by v/qk/gates/AV/cb/out-proj.
"""

import sys

if "/opt/trn_rl_repo" not in sys.path:
    sys.path.insert(0, "/opt/trn_rl_repo")

import numpy as np

import concourse.bass as bass
import concourse.tile as tile
from concourse import bacc, mybir
from concourse.bass_utils import run_bass_kernel_spmd

F32 = mybir.dt.float32
BF16 = mybir.dt.bfloat16
AF = mybir.ActivationFunctionType
ALU = mybir.AluOpType

B = 8
N = 1024
DIM = 512
HEADS = 8
DHEAD = 64
THETA = 10000.0
N_CORES = 8

NT = N // 128  # 8 token row tiles
KO = DIM // 128  # 4 contraction chunks
NC_ = N // 512  # 2 query column chunks of 512
MT = 4  # head pairs (2 heads x 64 dims = 128 partitions)
JC = 8  # key chunks of 128


def _rope_tables():
    """cos2T/sinS2T [128, N]: rows p = (h%2)*64 + d; identical per head half.

    sinS2T row 2t   = -sin(n * invf[t])  (multiplies shuffled value q[2t+1])
    sinS2T row 2t+1 = +sin(n * invf[t])
    """
    inv_freq = 1.0 / (THETA ** (np.arange(0, DHEAD, 2, dtype=np.float64) / DHEAD))
    pos = np.arange(N, dtype=np.float64)
    freqs = pos[None, :] * np.repeat(inv_freq, 2)[:, None]  # [64, N]
    cos = np.cos(freqs)
    sin = np.sin(freqs)
    sign = np.where(np.arange(DHEAD) % 2 == 0, -1.0, 1.0)[:, None]
    sin_signed = sin * sign
    cos2 = np.concatenate([cos, cos], axis=0).astype(np.float32)
    sin2 = np.concatenate([sin_signed, sin_signed], axis=0).astype(np.float32)
    return np.ascontiguousarray(cos2), np.ascontiguousarray(sin2)


def build_kernel():
    import ml_dtypes

    nc = bacc.Bacc("TRN2", target_bir_lowering=False, debug=False, num_devices=N_CORES)

    x_d = nc.dram_tensor("x", [N, DIM], F32, kind="ExternalInput").ap()
    gamma_d = nc.dram_tensor("gamma", [DIM], F32, kind="ExternalInput").ap()
    wqkv_d = nc.dram_tensor("w_qkv", [DIM, 3 * DIM], F32, kind="ExternalInput").ap()
    wg_d = nc.dram_tensor("w_g", [DIM, HEADS], F32, kind="ExternalInput").ap()
    bg_d = nc.dram_tensor("b_g", [HEADS], F32, kind="ExternalInput").ap()
    wo_d = nc.dram_tensor("w_o", [DIM, DIM], F32, kind="ExternalInput").ap()
    out_d = nc.dram_tensor("out", [N, DIM], F32, kind="ExternalOutput").ap()

    cos_np, sin_np = _rope_tables()
    cos_d = nc.inline_tensor(cos_np.astype(ml_dtypes.bfloat16), name="rope_cos").ap()
    sin_d = nc.inline_tensor(sin_np.astype(ml_dtypes.bfloat16), name="rope_sin").ap()
    eye_d = nc.inline_tensor(np.eye(128, dtype=np.float32), name="eye128").ap()

    # E[h, mt*128 + p] = 1 if head h owns partition p of pair-tile mt
    e_np = np.zeros((HEADS, MT * 128), np.float32)
    for mt in range(MT):
        for p in range(128):
            e_np[2 * mt + p // 64, mt * 128 + p] = 1.0
    e_d = nc.inline_tensor(e_np.astype(ml_dtypes.bfloat16), name="gate_bcast_e").ap()

    with tile.TileContext(nc) as tc:
        _build_tile(
            nc, tc, x_d, gamma_d, wqkv_d, wg_d, bg_d, wo_d, cos_d, sin_d, eye_d, e_d, out_d
        )

    nc.compile()
    return nc


def _build_tile(
    nc, tc, x_d, gamma_d, wqkv_d, wg_d, bg_d, wo_d, cos_d, sin_d, eye_d, e_d, out_d
):
    from contextlib import ExitStack

    ctx = ExitStack()
    with ctx:
        singles = ctx.enter_context(tc.tile_pool(name="singles", bufs=1))
        wpool = ctx.enter_context(tc.tile_pool(name="weights", bufs=1))
        wstage = ctx.enter_context(tc.tile_pool(name="wstage", bufs=1))
        xpool = ctx.enter_context(tc.tile_pool(name="x", bufs=1))
        xtbp = ctx.enter_context(tc.tile_pool(name="xtb", bufs=1))
        xtp = ctx.enter_context(tc.tile_pool(name="xhatT", bufs=1))
        qkpool = ctx.enter_context(tc.tile_pool(name="qk", bufs=1))
        vpool = ctx.enter_context(tc.tile_pool(name="vaug", bufs=1))
        spool = ctx.enter_context(tc.tile_pool(name="expS", bufs=18))
        gpool = ctx.enter_context(tc.tile_pool(name="gates", bufs=1))
        avpool = ctx.enter_context(tc.tile_pool(name="avg", bufs=1))
        scratch = ctx.enter_context(tc.tile_pool(name="scratch", bufs=2))

        # ---- x on two HWDGE rings, then gamma + w_qkv stages ----
        xts = [
            xpool.tile([128, DIM], F32, tag=f"xt{it % 5}", name=f"xt{it}")
            for it in range(NT)
        ]
        # x split across the two HWDGE rings, w_qkv stages on the Pool SWDGE
        # ring: 5MB of input on one queue was the binding prologue constraint.
        gamma_sb = singles.tile([128, KO], F32)
        nc.sync.dma_start(
            out=gamma_sb[:], in_=gamma_d.rearrange("(ko ki) -> ki ko", ki=128)
        )
        bg_sb = singles.tile([HEADS, 1], F32)
        nc.sync.dma_start(out=bg_sb[:], in_=bg_d.rearrange("(h o) -> h o", o=1))
        for it in range(NT):
            eng = nc.sync if it < 4 else nc.scalar
            eng.dma_start(out=xts[it][:], in_=x_d[it * 128:(it + 1) * 128, :])

        def _w_stage(ko):
            # Distinct stages (no buffer reuse): the v-column casts are
            # deferred past the prologue, so a reused stage would stall the
            # ko2/ko3 DMAs (and with them the ACT drain queue) behind them.
            return wstage.tile(
                [128, 3 * DIM], F32, tag=f"ws{ko}", name=f"ws{ko}", bufs=1
            )

        def _w_casts(ko, ws):
            # q|k columns: ACT/DVE right after the RMS stream drains;
            # v columns: Pool (first needed a window later)
            if ko % 2 == 0:
                nc.scalar.copy(out=wqkv_bf[:, ko, 0:2 * DIM], in_=ws[:, 0:2 * DIM])
            else:
                nc.vector.tensor_copy(
                    out=wqkv_bf[:, ko, 0:2 * DIM], in_=ws[:, 0:2 * DIM]
                )

        def _w_casts_v(ko, ws):
            # DVE, not gpsimd: the Pool ring must stay clear for the RoPE
            # sin-muls that gate the first score block.
            nc.vector.tensor_copy(
                out=wqkv_bf[:, ko, 2 * DIM:3 * DIM], in_=ws[:, 2 * DIM:3 * DIM]
            )

        # Preload the Sqrt act-table during DMA dead time: Square (every
        # table) then the real Sqrts would otherwise force a reload mid-RMS.
        warmt = scratch.tile([1, 1], F32, tag="warmt", name="warmt", bufs=1)
        nc.gpsimd.memset(warmt[:], 1.0)
        nc.scalar.activation(out=warmt[:], in_=warmt[:], func=AF.Sqrt)

        # ---- constants + w_qkv stages on the Pool SWDGE ring ----
        ident = singles.tile([128, 128], BF16)
        nc.gpsimd.dma_start(out=ident[:], in_=eye_d[:])
        wss = [_w_stage(0), _w_stage(1)]
        nc.gpsimd.dma_start(out=wss[0][:], in_=wqkv_d[0:128, :])
        nc.gpsimd.dma_start(out=wss[1][:], in_=wqkv_d[128:256, :])
        cosT = singles.tile([128, N], BF16)
        sinT = singles.tile([128, N], BF16)
        nc.gpsimd.dma_start(out=cosT[:], in_=cos_d[:])
        nc.gpsimd.dma_start(out=sinT[:], in_=sin_d[:])
        wss.append(_w_stage(2))
        nc.sync.dma_start(out=wss[2][:], in_=wqkv_d[256:384, :])
        wss.append(_w_stage(3))
        nc.scalar.dma_start(out=wss[3][:], in_=wqkv_d[384:512, :])
        wg_s = wpool.tile([128, KO, HEADS], F32)
        nc.gpsimd.dma_start(
            out=wg_s[:], in_=wg_d.rearrange("(ko ki) h -> ki ko h", ki=128)
        )
        wg_bf = wpool.tile([128, KO, HEADS], BF16)
        nc.gpsimd.tensor_copy(out=wg_bf[:], in_=wg_s[:])
        e_sb = singles.tile([HEADS, MT * 128], BF16, name="e_sb")
        nc.gpsimd.dma_start(out=e_sb[:], in_=e_d[:])

        # ---- RMSnorm -> xtb (bf16), 1-deep pipelined on ACT ----
        xhatT = [
            xtp.tile([128, N], BF16, tag=f"xhatT{ko}", name=f"xhatT{ko}")
            for ko in range(KO)
        ]
        xtb = [
            xtbp.tile([128, DIM], BF16, tag=f"xtb{it}", name=f"xtb{it}")
            for it in range(NT)
        ]
        sss = []

        def _rms_head(it):
            ss = scratch.tile([128, 1], F32, tag=f"ss{it}", name=f"ss{it}", bufs=1)
            # Square's elementwise output is unused; scribble it into xtb[it],
            # which the tensor_scalar_mul in _rms_tail overwrites.
            nc.scalar.activation(
                out=xtb[it][:], in_=xts[it][:], func=AF.Square, accum_out=ss[:]
            )
            sss.append(ss)

        def _rms_tail(it):
            nc.scalar.activation(
                out=sss[it][:], in_=sss[it][:], func=AF.Sqrt, scale=1.0 / DIM
            )
            nc.vector.reciprocal(out=sss[it][:], in_=sss[it][:])
            nc.vector.tensor_scalar_mul(
                out=xtb[it][:], in0=xts[it][:], scalar1=sss[it][:]
            )

        _rms_head(0)
        for it in range(1, NT):
            _rms_head(it)
            _rms_tail(it - 1)
        _rms_tail(NT - 1)
        # Warm the Exp table set now (ACT is DMA-bound here): the drain
        # Copies don't switch sets, so ge/score exps then hit a warm table
        # instead of reloading on the first-exp critical path.
        nc.scalar.activation(out=warmt[:], in_=warmt[:], func=AF.Exp)

        negbg_sb = singles.tile([HEADS, 1], F32)
        nc.vector.tensor_scalar_mul(out=negbg_sb[:], in0=bg_sb[:], scalar1=-1.0)
        wqkv_bf = wpool.tile([128, KO, 3 * DIM], BF16)
        _w_casts(0, wss[0])
        _w_casts(1, wss[1])
        _w_casts(2, wss[2])
        _w_casts(3, wss[3])
        # v-column casts + v_aug memsets + w_o load/cast are deferred into
        # the window schedule below: nothing on the Pool/DVE rings may delay
        # the first RoPE ops, which gate the first exp.

        # ---- persistent SBUF for attention ----
        v_aug = [
            vpool.tile([128, HEADS * 128], BF16, tag=f"va{it}", name=f"va{it}")
            for it in range(NT)
        ]

        def va_memset(it):
            va3 = v_aug[it][:].rearrange("p (q c) -> p q c", q=HEADS // 2)
            nc.gpsimd.memset(va3[:, :, 64:192], 1.0)

        wo_st = wpool.tile([128, KO, DIM], F32)
        wo_sb = wpool.tile([128, KO, DIM], BF16)

        def wo_load():
            nc.gpsimd.dma_start(
                out=wo_st[:], in_=wo_d.rearrange("(ko ki) d -> ki ko d", ki=128)
            )

        def wo_cast(ko):
            nc.gpsimd.tensor_copy(out=wo_sb[:, ko, :], in_=wo_st[:, ko, :])

        qT = [qkpool.tile([128, N], BF16, tag=f"q{mt}", name=f"q{mt}") for mt in range(MT)]
        kT = [qkpool.tile([128, N], BF16, tag=f"k{mt}", name=f"k{mt}") for mt in range(MT)]
        avg = [
            avpool.tile([128, N], BF16, tag=f"avg{mt}", name=f"avg{mt}")
            for mt in range(MT)
        ]
        gT = gpool.tile([HEADS, N], F32)
        denomW = gpool.tile([HEADS, N], BF16)
        cT = gpool.tile([HEADS, N], BF16)

        shuf_mask = [(i ^ 1) for i in range(32)]
        scale = 1.0 / float(np.sqrt(DHEAD))

        es = [[[None for _ in range(2)] for _ in range(JC)] for _ in range(MT)]
        avraw = {}

        with (
            tc.tile_pool(name="ps_s", bufs=2, space="PSUM") as ps_s,
            tc.tile_pool(name="ps_u", bufs=2, space="PSUM") as ps_u,
        ):
            def mm(out_ps, lhsT, rhs, start, stop):
                nc.tensor.matmul(out_ps, lhsT, rhs, start=start, stop=stop)

            def u_tile(nm):
                return ps_u.tile([128, N], F32, tag="u", name=nm)

            # ---- PE transpose of xtb -> xhatT through the S psum banks
            # (bf16 view of the f32 tiles); gamma folds into the drains ----
            for ic in range(NC_):
                trp = ps_s.tile([128, N], F32, tag="sps", name=f"tr{ic}")
                trv = trp[:].bitcast(BF16).rearrange("p (ko c) -> p ko c", ko=KO)
                for s in range(4):
                    it = ic * 4 + s
                    for ko in range(KO):
                        nc.tensor.transpose(
                            trv[:, ko, s * 128:(s + 1) * 128],
                            xtb[it][:, ko * 128:(ko + 1) * 128],
                            ident[:],
                        )
                for ko in range(KO):
                    dst = xhatT[ko][:, ic * 512:(ic + 1) * 512]
                    if ko % 2 == 0:
                        nc.scalar.activation(
                            out=dst,
                            in_=trv[:, ko, :],
                            func=AF.Copy,
                            scale=gamma_sb[:, ko:ko + 1],
                        )
                    else:
                        nc.vector.tensor_scalar_mul(
                            out=dst,
                            in0=trv[:, ko, :],
                            scalar1=gamma_sb[:, ko:ko + 1],
                        )

            def gates_block():
                # sigmoid(z) = 1/(1+exp(-z)) — keeps ACT on the Exp table set
                g_ps = u_tile("gps")
                for ko in range(KO):
                    for ic in range(NC_):
                        mm(
                            g_ps[0:HEADS, ic * 512:(ic + 1) * 512],
                            wg_bf[:, ko, :],
                            xhatT[ko][:, ic * 512:(ic + 1) * 512],
                            start=(ko == 0),
                            stop=(ko == KO - 1),
                        )
                ge = scratch.tile([HEADS, N], F32, tag="ge", name="ge", bufs=1)
                nc.scalar.activation(
                    out=ge[:],
                    in_=g_ps[0:HEADS, :],
                    func=AF.Exp,
                    scale=-1.0,
                    bias=negbg_sb[:],
                )
                nc.vector.tensor_scalar_add(out=ge[:], in0=ge[:], scalar1=1.0)
                nc.vector.reciprocal_approx_fast(out=gT[:], in_=ge[:])

            def v_dual(it0):
                """Project v row-tiles it0, it0+1 into one dual psum tile."""
                vps = u_tile(f"vps{it0}")
                for half, it in enumerate((it0, it0 + 1)):
                    hs = slice(half * 512, half * 512 + 512)
                    for ko in range(KO):
                        mm(
                            vps[:, hs],
                            xhatT[ko][:, it * 128:(it + 1) * 128],
                            wqkv_bf[:, ko, 2 * DIM:3 * DIM],
                            start=(ko == 0),
                            stop=(ko == KO - 1),
                        )
                # src col q*128+s*64+d -> dst col q*256 + s*192 + d
                for half, it in enumerate((it0, it0 + 1)):
                    vsrc = vps[:, half * 512:half * 512 + 512].rearrange(
                        "p (q s d) -> p q s d", q=4, s=2
                    )
                    vdst = v_aug[it][:].rearrange("p (q c) -> p q c", q=4)
                    nc.vector.tensor_copy(out=vdst[:, :, 0:64], in_=vsrc[:, :, 0, :])
                    nc.vector.tensor_copy(out=vdst[:, :, 192:256], in_=vsrc[:, :, 1, :])

            def qk_block(mt, which):
                """Project+RoPE the full [128, N] qT/kT for pair mt (dual-ic)."""
                dest = qT if which == 0 else kT
                col0 = which * DIM + mt * 128
                pps = u_tile(f"pps{mt}{which}")
                for ko in range(KO):
                    for ic in range(NC_):
                        mm(
                            pps[:, ic * 512:(ic + 1) * 512],
                            wqkv_bf[:, ko, col0:col0 + 128],
                            xhatT[ko][:, ic * 512:(ic + 1) * 512],
                            start=(ko == 0),
                            stop=(ko == KO - 1),
                        )
                shuf = scratch.tile([128, N], F32, tag="shuf", name="shuf", bufs=2)
                nc.vector.stream_shuffle(shuf[:], pps[:], mask=shuf_mask)
                t1 = scratch.tile([128, N], BF16, tag="rt1", name="rt1", bufs=2)
                nc.vector.tensor_tensor(
                    out=t1[:], in0=pps[:], in1=cosT[:], op=ALU.mult
                )
                t2 = scratch.tile([128, N], BF16, tag="rt2", name="rt2", bufs=2)
                nc.gpsimd.tensor_tensor(
                    out=t2[:], in0=shuf[:], in1=sinT[:], op=ALU.mult
                )
                nc.vector.tensor_tensor(
                    out=dest[mt][:], in0=t1[:], in1=t2[:], op=ALU.add
                )

            def s_block(mt, jc, hh):
                """Scores S^T [128 keys, N queries] for (pair, key chunk, head) + exp."""
                pr = slice(hh * 64, (hh + 1) * 64)
                sp = ps_s.tile([128, N], F32, tag="sps", name="sps")
                for ic in range(NC_):
                    mm(
                        sp[:, ic * 512:(ic + 1) * 512],
                        kT[mt][pr, jc * 128:(jc + 1) * 128],
                        qT[mt][pr, ic * 512:(ic + 1) * 512],
                        start=True,
                        stop=True,
                    )
                e = spool.tile([128, N], BF16, tag="es", name="es")
                nc.scalar.activation(out=e[:], in_=sp[:], func=AF.Exp, scale=scale)
                es[mt][jc][hh] = e

            def av_open(mt, hh):
                if mt not in avraw:
                    avraw[mt] = avpool.tile(
                        [128, 2 * N], BF16, tag="avraw", name=f"avraw{mt}", bufs=2
                    )
                return u_tile(f"avp{mt}{hh}")

            def av_chunks(t, mt, hh, jcs):
                """Dual-ic AV accumulation chunks: one stationary per key chunk."""
                h = 2 * mt + hh
                base = (h // 2) * 256 + (h % 2) * 128
                for jc in jcs:
                    for ic in range(NC_):
                        mm(
                            t[:, ic * 512:(ic + 1) * 512],
                            v_aug[jc][:, base:base + 128],
                            es[mt][jc][hh][:, ic * 512:(ic + 1) * 512],
                            start=(jc == 0),
                            stop=(jc == JC - 1),
                        )

            def av_drain(t, mt, hh, drain_act=False):
                dst = avraw[mt][:, hh * N:(hh + 1) * N]
                if drain_act:
                    nc.scalar.copy(out=dst, in_=t[:])
                else:
                    nc.vector.tensor_copy(out=dst, in_=t[:])

            def denom_even(mt):
                nc.sync.dma_start(
                    out=denomW[2 * mt:2 * mt + 1, :],
                    in_=avraw[mt][64:65, 0:N],
                )

            def denom_odd(mt):
                nc.sync.dma_start(
                    out=denomW[2 * mt + 1:2 * mt + 2, :],
                    in_=avraw[mt][0:1, N:2 * N],
                )

            def gate_recip(mt, isl=slice(0, N)):
                # DVE partition bases must be 32-aligned: process all 8 head
                # rows (cost is free-size-bound); stale rows are finite (the
                # denomW memset) and meet zero E-columns in cb_avg.
                # recip_approx needs fp32, denomW is bf16: widen first.
                dwf = scratch.tile([HEADS, N], F32, tag="ge", name="dwf", bufs=1)
                nc.vector.tensor_copy(out=dwf[:, isl], in_=denomW[:, isl])
                dscr = scratch.tile([HEADS, N], F32, tag="dscr", name="dscr", bufs=1)
                nc.vector.reciprocal_approx_fast(out=dscr[:, isl], in_=dwf[:, isl])
                nc.vector.tensor_tensor(
                    out=cT[:, isl], in0=dscr[:, isl], in1=gT[:, isl], op=ALU.mult
                )

            def cb_avg(mt, ics=(0, 1)):
                """Broadcast gate/denominator rows to pair partitions; scale AV."""
                cb_ps = u_tile(f"cbps{mt}")
                for ic in ics:
                    mm(
                        cb_ps[:, ic * 512:(ic + 1) * 512],
                        e_sb[:, mt * 128:(mt + 1) * 128],
                        cT[:, ic * 512:(ic + 1) * 512],
                        start=True,
                        stop=True,
                    )
                cbb = scratch.tile([128, N], BF16, tag="cbb", name="cbb", bufs=1)
                for ic in ics:
                    isl = slice(ic * 512, (ic + 1) * 512)
                    nc.vector.tensor_copy(out=cbb[:, isl], in_=cb_ps[:, isl])
                    nc.vector.tensor_tensor(
                        out=avg[mt][0:64, isl],
                        in0=avraw[mt][0:64, isl],
                        in1=cbb[0:64, isl],
                        op=ALU.mult,
                    )
                    nc.vector.tensor_tensor(
                        out=avg[mt][64:128, isl],
                        in0=avraw[mt][64:128, N + ic * 512:N + (ic + 1) * 512],
                        in1=cbb[64:128, isl],
                        op=ALU.mult,
                    )

            def out_block(it):
                ops = u_tile(f"ops{it}")
                for mt in range(MT):
                    mm(
                        ops[:, 0:512],
                        avg[mt][:, it * 128:(it + 1) * 128],
                        wo_sb[:, mt, :],
                        start=(mt == 0),
                        stop=(mt == MT - 1),
                    )
                osb = scratch.tile([128, DIM], F32, tag="osb", name="osb", bufs=3)
                if it % 2 == 0:
                    nc.scalar.copy(out=osb[:], in_=ops[:, 0:512])
                else:
                    nc.vector.tensor_copy(out=osb[:], in_=ops[:, 0:512])
                eng = (nc.sync, nc.scalar, nc.gpsimd)[it % 3]
                eng.dma_start(out=out_d[it * 128:(it + 1) * 128, :], in_=osb[:])

            # ---- prologue: qk of pair 0, deferred v casts, then gates
            # (gates' DVE ops wait on ACT's ge exp; casts must not queue
            # behind that stall) ----
            qk_block(0, 0)
            qk_block(0, 1)
            for ko in range(KO):
                _w_casts_v(ko, wss[ko])
            gates_block()

            # ---- window 0: scores(0) + v projections + qk(1) ----
            for jc in range(JC):
                s_block(0, jc, 0)
                s_block(0, jc, 1)
                if jc == 0:
                    va_memset(6)
                    va_memset(7)
                elif jc == 1:
                    v_dual(0)
                    va_memset(4)
                    va_memset(5)
                elif jc == 2:
                    v_dual(2)
                    va_memset(2)
                    va_memset(3)
                elif jc == 3:
                    qk_block(1, 0)
                elif jc == 4:
                    v_dual(4)
                    va_memset(0)
                    va_memset(1)
                elif jc == 5:
                    qk_block(1, 1)
                elif jc == 6:
                    v_dual(6)
                    nc.gpsimd.memset(denomW[:], 1.0)

            # ---- windows 1, 2: scores(w) + AV(w-1) + qk(w+1) + postproc(w-2) ----
            for w in (1, 2):
                avA = avB = None
                for jc in range(JC):
                    s_block(w, jc, 0)
                    s_block(w, jc, 1)
                    if jc == 0:
                        avA = av_open(w - 1, 0)
                        av_chunks(avA, w - 1, 0, (0, 1))
                        if w == 1:
                            wo_load()
                    elif jc == 1:
                        av_chunks(avA, w - 1, 0, (2, 3))
                        qk_block(w + 1, 0)
                    elif jc == 2:
                        av_chunks(avA, w - 1, 0, (4, 5))
                        if w >= 2:
                            gate_recip(w - 2)
                        else:
                            wo_cast(0)
                            wo_cast(1)
                    elif jc == 3:
                        av_chunks(avA, w - 1, 0, (6, 7))
                        av_drain(avA, w - 1, 0)
                        denom_even(w - 1)
                        if w >= 2:
                            cb_avg(w - 2)
                    elif jc == 4:
                        avB = av_open(w - 1, 1)
                        av_chunks(avB, w - 1, 1, (0, 1))
                        if w == 1:
                            wo_cast(2)
                            wo_cast(3)
                    elif jc == 5:
                        av_chunks(avB, w - 1, 1, (2, 3))
                        qk_block(w + 1, 1)
                    elif jc == 6:
                        av_chunks(avB, w - 1, 1, (4, 5))
                    elif jc == 7:
                        av_chunks(avB, w - 1, 1, (6, 7))
                        av_drain(avB, w - 1, 1)
                        denom_odd(w - 1)

            # ---- window 3, sub-window A: all hh0 scores + AV(2) + postproc(1) ----
            avA = avB = None
            for jc in range(JC):
                s_block(3, jc, 0)
                if jc == 0:
                    avA = av_open(2, 0)
                    av_chunks(avA, 2, 0, (0, 1))
                elif jc == 1:
                    av_chunks(avA, 2, 0, (2, 3))
                elif jc == 2:
                    av_chunks(avA, 2, 0, (4, 5))
                    gate_recip(1)
                elif jc == 3:
                    av_chunks(avA, 2, 0, (6, 7))
                    av_drain(avA, 2, 0)
                    denom_even(2)
                    cb_avg(1)
                elif jc == 4:
                    avB = av_open(2, 1)
                    av_chunks(avB, 2, 1, (0, 1))
                elif jc == 5:
                    av_chunks(avB, 2, 1, (2, 3))
                elif jc == 6:
                    av_chunks(avB, 2, 1, (4, 5))
                elif jc == 7:
                    av_chunks(avB, 2, 1, (6, 7))
                    av_drain(avB, 2, 1)
                    denom_odd(2)

            # ---- window 3, sub-window B: all hh1 scores, AV(3,hh0) early,
            # AV(3,hh1) chunks as their exps land, postproc(2) ----
            av3A = av3B = None
            for jc in range(JC):
                s_block(3, jc, 1)
                if jc == 0:
                    av3A = av_open(3, 0)
                    av_chunks(av3A, 3, 0, (0, 1, 2, 3))
                elif jc == 1:
                    av_chunks(av3A, 3, 0, (4, 5, 6, 7))
                    gate_recip(2)
                elif jc == 2:
                    av_drain(av3A, 3, 0)
                    denom_even(3)
                    cb_avg(2)
                elif jc == 3:
                    av3B = av_open(3, 1)
                    av_chunks(av3B, 3, 1, (0, 1))
                elif jc >= 4:
                    av_chunks(av3B, 3, 1, (jc - 2,))

            # ---- tail: last AV chunks, split postproc(3), out-projection ----
            av_chunks(av3B, 3, 1, (6, 7))
            av_drain(av3B, 3, 1, drain_act=True)
            denom_odd(3)
            gate_recip(3, slice(0, 512))
            cb_avg(3, ics=(0,))
            for it in range(4):
                out_block(it)
            gate_recip(3, slice(512, N))
            cb_avg(3, ics=(1,))
            for it in range(4, NT):
                out_block(it)


_NC_CACHE = None


def _get_nc():
    global _NC_CACHE
    if _NC_CACHE is None:
        _NC_CACHE = build_kernel()
    return _NC_CACHE


def kernel(**inputs):
    x = np.ascontiguousarray(np.asarray(inputs["x"], dtype=np.float32))
    gamma = np.ascontiguousarray(np.asarray(inputs["gamma"], dtype=np.float32))
    w_qkv = np.ascontiguousarray(np.asarray(inputs["w_qkv"], dtype=np.float32))
    w_g = np.ascontiguousarray(np.asarray(inputs["w_g"], dtype=np.float32))
    b_g = np.ascontiguousarray(np.asarray(inputs["b_g"], dtype=np.float32))
    w_o = np.ascontiguousarray(np.asarray(inputs["w_o"], dtype=np.float32))

    nc = _get_nc()
    in_maps = []
    for i in range(N_CORES):
        in_maps.append(
            {
                "x": np.ascontiguousarray(x[i]),
                "gamma": gamma,
                "w_qkv": w_qkv,
                "w_g": w_g,
                "b_g": b_g,
                "w_o": w_o,
            }
        )
    res = run_bass_kernel_spmd(nc, in_maps, core_ids=list(range(N_CORES)))
    out = np.stack([res.results[i]["out"] for i in range(N_CORES)], axis=0)
    return out.astype(np.float32)


if __name__ == "__main__":
    rng = np.random.default_rng(0)
    ins = {
        "x": rng.standard_normal((B, N, DIM), dtype=np.float32),
        "gamma": np.ones((DIM,), np.float32),
        "w_qkv": (rng.standard_normal((DIM, 3 * DIM), dtype=np.float32) / np.sqrt(DIM)),
        "w_g": (rng.standard_normal((DIM, HEADS), dtype=np.float32) / np.sqrt(DIM)),
        "b_g": np.zeros((HEADS,), np.float32),
        "w_o": (rng.standard_normal((DIM, DIM), dtype=np.float32) / np.sqrt(DIM)),
    }
    out = kernel(**ins)
    print("out", out.shape, out.dtype, float(np.abs(out).mean()))
